# revision 1
# baseline (speedup 1.0000x reference)
"""Multi-head attention (S=4096, D=1024, H=16) on 8 trn2 NeuronCores.

Sharding: 2 heads per core (tensor-parallel on Q/K/V column splits and
dense row split). Each core computes a partial [S, D] output; host sums
the 8 partials (the unshard step for row-parallel TP).

Per-core layout (all fp32; matmuls run as float32r, N>=256):
  inputs : xT  [1024, 4096]  x transposed (same array on every core)
           wqT/wkT/wvT [1024, 128]  W[shard_rows].T for this core's 2 heads
           woT [128, 1024]          Wo[:, shard_cols].T
           ident [128, 128]         identity for PE transposes
  output : y [4096, 1024]           partial output

Per-core dataflow (heads h in {0,1}, dk=64):
  qT/kT [128(2h x 64d), 4096s]; v via vT chunks + PE transpose into
  v_aug [t-block, h, 65] (col 64 = ones -> softmax denominator rides the
  PV matmul). 8 s-waves of 512:
    LT[t, s0:s0+512] = k q^T per 128-t-block (K=64 row groups, both heads
    in one [128,1024] psum tile), PT = exp(0.125*LT) on ACT (no max
    subtraction: logits ~ N(0,1)), attnT_aug[65,512] += v_aug.T @ PT.
  Wave finalize: den -> reciprocal -> K=1 broadcast matmul -> normalize
  fused into the attnT copy -> y = attnT.T @ woT -> DMA out. PV and
  finalize run through a lagged FIFO so the in-order PE queue never
  stalls on unreleased PSUM slots or DVE-latency chains; the first 12
  attention iterations interleave with the projection phase.
"""

import numpy as np
from collections import deque
from contextlib import ExitStack

S = 4096
D = 1024
NCORES = 8
HD = 128  # head-dim span per core (2 heads x 64)
DK = 64

_NC_CACHE = {}


def _split_multi_waits(nc, mybir):
    """This walrus build encodes at most ~2 sync commands per instruction
    (1 for matmul/drain). Keep <=1 wait on every compute/DMA instruction and
    move the rest into standalone dual-condition EventSemaphore instructions
    inserted immediately before it on the same engine (same wait point, so
    semantics are unchanged)."""
    n = 0
    used = set()
    for b in nc.m.functions[0].blocks:
        for inst in b.instructions:
            si = inst.sync_info
            if si:
                for w in (si.on_wait or []):
                    used.add(w.id)
                for u in (si.on_update or []):
                    used.add(u.id)
    free_ids = [i for i in range(max(used) + 1, max(used) + 32)]
    sems = {}

    def eng_sem(eng):
        if eng not in sems:
            sems[eng] = (free_ids.pop(0), f"wsplit_{len(sems)}")
        return sems[eng]

    for b in nc.m.functions[0].blocks:
        il = b.instructions
        new = []
        for inst in il:
            si = inst.sync_info
            waits = list(si.on_wait) if si and si.on_wait else []
            upds = list(si.on_update) if si and si.on_update else []
            if type(inst).__name__ == "InstEventSemaphore":
                new.append(inst)
                continue
            if len(waits) > 1:
                excess, keep = waits[:-1], waits[-1:]
                for i in range(0, len(excess), 2):
                    sid, sname = eng_sem(inst.engine)
                    ev = mybir.InstEventSemaphore(
                        name=f"{inst.name}_ws{i}", engine=inst.engine,
                        ins=[], outs=[],
                        sync_info=mybir.SyncInfo(
                            on_wait=excess[i:i + 2],
                            on_update=[mybir.SyncUpdate(
                                sync_type="semaphore", id=sid,
                                ant_name=sname, update_mode="sem-inc",
                                update_value=1, update_reg=None)]))
                    new.append(ev)
                    n += 1
                inst.sync_info = mybir.SyncInfo(on_wait=keep, on_update=upds)
            new.append(inst)
        il[:] = new
    return n


def _build_nc():
    import concourse.bass as bass
    import concourse.tile as tile
    import concourse.mybir as mybir

    fp32 = mybir.dt.float32
    f32r = mybir.dt.float32r
    AF = mybir.ActivationFunctionType

    nc = bass.Bass()
    xT = nc.dram_tensor("xT", [D, S], f32r, kind="ExternalInput")
    wqT = nc.dram_tensor("wqT", [D, HD], f32r, kind="ExternalInput")
    wkT = nc.dram_tensor("wkT", [D, HD], f32r, kind="ExternalInput")
    wvT = nc.dram_tensor("wvT", [D, HD], f32r, kind="ExternalInput")
    woT = nc.dram_tensor("woT", [HD, D], f32r, kind="ExternalInput")
    ident = nc.dram_tensor("ident", [128, 128], f32r, kind="ExternalInput")
    ones_in = nc.dram_tensor("ones_in", [128, DK], f32r, kind="ExternalInput")
    y = nc.dram_tensor("y", [S, D], fp32, kind="ExternalOutput")

    NT = S // 128   # 32 t-blocks
    NWV = 8         # s-waves of 512

    with tile.TileContext(nc) as tc, ExitStack() as ctx, \
         nc.allow_low_precision(reason="float32r matmul operand rounding"):
        sb = ctx.enter_context(tc.tile_pool(name="sb", bufs=1))

        qT_sb = sb.tile([128, S], f32r, tag="qT")
        kT_sb = sb.tile([128, S], f32r, tag="kT")
        # v_aug[:, tb, h, 0:64] = v block for head h; [..., 64] = ones
        v_aug = sb.tile([128, NT, 2, DK + 1], f32r, tag="vaug")
        attnT = sb.tile([128, S], f32r, tag="attnT")
        ones_r = sb.tile([1, DK], f32r, tag="onesr")
        warm = sb.tile([1, DK], fp32, tag="warm")
        wo_sb = sb.tile([HD, D], f32r, tag="wo")
        id_sb = sb.tile([128, 128], f32r, tag="id")

        nc.sync.dma_start(wo_sb[:], woT[:])
        nc.sync.dma_start(id_sb[:], ident[:])
        # f32r constants must come from DRAM (memset cannot round to f32r)
        nc.sync.dma_start(ones_r[:], ones_in[0:1, :])
        nc.sync.dma_start(
            v_aug[:, :, :, DK],
            ones_in[:, 0:DK].rearrange("p (a b) -> p a b", a=NT))
        # preload the exp table set on ACT before the first real exp
        nc.scalar.activation(warm[:], ones_r[:], AF.Exp, scale=0.125)

        proj_ctx = ExitStack()
        with tc.tile_pool(name="wpool", bufs=1) as wpool, \
             tc.tile_pool(name="xpool", bufs=2) as xpool, \
             tc.tile_pool(name="ltpool", bufs=2, space="PSUM") as ltpool, \
             tc.tile_pool(name="ptpool", bufs=12) as ptpool, \
             tc.tile_pool(name="opool", bufs=3) as opool, \
             tc.tile_pool(name="dpool", bufs=2) as dpool:
            ppsum = proj_ctx.enter_context(
                tc.tile_pool(name="ppsum", bufs=3, space="PSUM"))
            tpsum = proj_ctx.enter_context(
                tc.tile_pool(name="tpsum", bufs=1, space="PSUM"))

            wq_sb = wpool.tile([128, 8, HD], f32r, tag="wq")
            wk_sb = wpool.tile([128, 8, HD], f32r, tag="wk")
            wv_sb = wpool.tile([128, 8, HD], f32r, tag="wv")
            nc.sync.dma_start(wq_sb[:], wqT.rearrange("(c p) m -> p c m", p=128))
            nc.sync.dma_start(wk_sb[:], wkT.rearrange("(c p) m -> p c m", p=128))
            nc.sync.dma_start(wv_sb[:], wvT.rearrange("(c p) m -> p c m", p=128))
            xTr = xT.rearrange("(c p) s -> p c s", p=128)

            # ---------- projections for one half-quarter (s/t range p*512) ----
            def proj_piece(p):
                xq = xpool.tile([128, 8, 512], f32r, tag="xq", name=f"xq_{p}")
                for c in range(0, 8, 2):
                    nc.sync.dma_start(xq[:, c:c + 2, :],
                                      xTr[:, c:c + 2, p * 512:(p + 1) * 512])
                s0 = p * 512
                pk = ppsum.tile([128, 512], fp32, tag="proj", name=f"pk_{p}")
                for c in range(8):
                    nc.tensor.matmul(pk[:], wk_sb[:, c, :], xq[:, c, :],
                                     start=(c == 0), stop=(c == 7))
                nc.any.tensor_copy(kT_sb[:, s0:s0 + 512], pk[:])
                pq = ppsum.tile([128, 512], fp32, tag="proj", name=f"pq_{p}")
                for c in range(8):
                    nc.tensor.matmul(pq[:], wq_sb[:, c, :], xq[:, c, :],
                                     start=(c == 0), stop=(c == 7))
                nc.any.tensor_copy(qT_sb[:, s0:s0 + 512], pq[:])
                pvT = ppsum.tile([128, 512], fp32, tag="proj", name=f"pv_{p}")
                for c in range(8):
                    nc.tensor.matmul(pvT[:], wv_sb[:, c, :], xq[:, c, :],
                                     start=(c == 0), stop=(c == 7))
                vtmp = xpool.tile([128, 512], f32r, tag="vtmp", name=f"vtmp_{p}")
                nc.any.tensor_copy(vtmp[:], pvT[:])
                for i in range(4):
                    tb = p * 4 + i
                    pvt = tpsum.tile([128, 128], f32r, tag="projvt",
                                     name=f"pvt_{tb}")
                    nc.tensor.transpose(pvt[:],
                                        vtmp[:, i * 128:(i + 1) * 128],
                                        id_sb[:])
                    nc.any.tensor_copy(
                        v_aug[:, tb, :, 0:DK],
                        pvt[:].rearrange("p (h d) -> p h d", h=2))

            # ---------- attention machinery ----------
            LAG = 4
            pending = deque()  # (ready_gi, is_pv, thunk)
            state = {"gi": 0, "maxpv": 0}
            lt_holder = {}

            def emit_lt(w, tb):
                s0 = w * 512
                lt = ltpool.tile([128, 1024], fp32, tag="lt",
                                 name=f"lt_{w}_{tb}")
                for h in range(2):
                    nc.tensor.matmul(
                        lt[:, h * 512:(h + 1) * 512],
                        kT_sb[DK * h:DK * (h + 1), tb * 128:(tb + 1) * 128],
                        qT_sb[DK * h:DK * (h + 1), s0:s0 + 512],
                        start=True, stop=True,
                        tile_position=(DK * h, 0),
                    )
                return lt

            def pv_thunk(w, tb, pt, accs):
                def run():
                    if tb == 0:
                        accs.extend(
                            accpool.tile([DK + 1, 512], fp32, tag="acc",
                                         name=f"acc_{w}_{h}") for h in range(2))
                    for h in range(2):
                        nc.tensor.matmul(
                            accs[h][:],
                            v_aug[:, tb, h, :],
                            pt[:, h * 512:(h + 1) * 512],
                            start=(tb == 0), stop=(tb == NT - 1),
                        )
                return run

            def finalize_thunks(w, accs):
                s0 = w * 512
                denw = dpool.tile([1, 1024], fp32, tag="denw", name=f"den_{w}")
                rdenw = dpool.tile([1, 1024], f32r, tag="rdenw", name=f"rden_{w}")

                def den_recip():
                    for h in range(2):
                        nc.vector.tensor_copy(denw[0:1, h * 512:(h + 1) * 512],
                                              accs[h][DK:DK + 1, :])
                    nc.vector.reciprocal(rdenw[:], denw[:])

                def norm(h):
                    bc = ypool.tile([DK, 512], fp32, tag="y", name=f"bc_{w}_{h}")
                    nc.tensor.matmul(
                        bc[:], ones_r[:],
                        rdenw[0:1, h * 512:(h + 1) * 512],
                        start=True, stop=True)
                    # walrus rejects tensor_tensor with two PSUM operands
                    bcs = opool.tile([DK, 512], fp32, tag="bcs",
                                     name=f"bcs_{w}_{h}")
                    nc.vector.tensor_copy(bcs[:], bc[:])
                    nc.vector.tensor_mul(attnT[DK * h:DK * (h + 1), s0:s0 + 512],
                                         accs[h][0:DK, :], bcs[:])

                def yblock(bl, jc):
                    b = w * 4 + bl
                    yp = ypool.tile([128, 512], fp32, tag="y",
                                    name=f"yp_{b}_{jc}")
                    nc.tensor.matmul(
                        yp[:],
                        attnT[:, b * 128:(b + 1) * 128],
                        wo_sb[:, jc * 512:(jc + 1) * 512],
                        start=True, stop=True)
                    yo = opool.tile([128, 512], fp32, tag="yo",
                                    name=f"yo_{b}_{jc}")
                    nc.vector.tensor_copy(yo[:], yp[:])
                    nc.sync.dma_start(
                        y[b * 128:(b + 1) * 128, jc * 512:(jc + 1) * 512],
                        yo[:])

                thunks = [den_recip, lambda: norm(0), lambda: norm(1)]
                for bl in range(4):
                    for jc in range(2):
                        thunks.append(lambda bl=bl, jc=jc: yblock(bl, jc))
                return thunks

            def emit_iter(w, tb, accs, ready_floor=0):
                gi = state["gi"]
                lt = lt_holder.pop("lt")
                pt = ptpool.tile([128, 1024], f32r, tag="pt",
                                 name=f"pt_{w}_{tb}")
                nc.scalar.activation(pt[:], lt[:], AF.Exp, scale=0.125)
                if tb + 1 < NT:
                    lt_holder["lt"] = emit_lt(w, tb + 1)
                elif w + 1 < NWV:
                    lt_holder["lt"] = emit_lt(w + 1, 0)
                pending.append((max(gi + LAG, ready_floor), True,
                                pv_thunk(w, tb, pt, accs)))
                state["maxpv"] = max(state["maxpv"],
                                     sum(1 for e in pending if e[1]))
                pops = 0
                while pending and pending[0][0] <= gi and pops < 2:
                    pending.popleft()[2]()
                    pops += 1
                state["gi"] = gi + 1

            # ---------- emission: proj with first attention iters woven in ----
            proj_piece(0)
            proj_piece(1)
            lt_holder["lt"] = emit_lt(0, 0)
            acc0 = []
            tb0 = 0
            for p in range(2, 8):
                proj_piece(p)
                emit_iter(0, tb0, acc0, ready_floor=8)
                tb0 += 1
            proj_ctx.close()
            att_ctx = ExitStack()
            accpool = att_ctx.enter_context(
                tc.tile_pool(name="accpool", bufs=3, space="PSUM"))
            ypool = att_ctx.enter_context(
                tc.tile_pool(name="ypool", bufs=1, space="PSUM"))
            for tb in range(tb0, NT):
                emit_iter(0, tb, acc0)
            for t in finalize_thunks(0, acc0):
                pending.append((state["gi"] + LAG - 1, False, t))

            for w in range(1, NWV):
                accs = []
                for tb in range(NT):
                    emit_iter(w, tb, accs)
                for t in finalize_thunks(w, accs):
                    pending.append((state["gi"] + LAG - 1, False, t))
            while pending:
                pending.popleft()[2]()
            assert state["maxpv"] <= 10, f"pt pool too small: {state['maxpv']}"
            att_ctx.close()

    _split_multi_waits(nc, mybir)
    nc.finalize()
    return nc


def _get_nc():
    if "nc" not in _NC_CACHE:
        _NC_CACHE["nc"] = _build_nc()
    return _NC_CACHE["nc"]


def _in_maps(x, Wq, Wk, Wv, Wo):
    xT = np.ascontiguousarray(x.T).astype(np.float32, copy=False)
    ident = np.eye(128, dtype=np.float32)
    maps = []
    for c in range(NCORES):
        sl = slice(HD * c, HD * (c + 1))
        maps.append(dict(
            xT=xT,
            wqT=np.ascontiguousarray(Wq[sl, :].T),
            wkT=np.ascontiguousarray(Wk[sl, :].T),
            wvT=np.ascontiguousarray(Wv[sl, :].T),
            woT=np.ascontiguousarray(Wo[:, sl].T),
            ident=ident,
            ones_in=np.ones((128, DK), dtype=np.float32),
        ))
    return maps


def kernel(x, Wq, Wk, Wv, Wo):
    from concourse.bass_utils import run_bass_kernel_spmd

    x = np.asarray(x, dtype=np.float32)
    nc = _get_nc()
    res = run_bass_kernel_spmd(nc, _in_maps(x, Wq, Wk, Wv, Wo),
                               list(range(NCORES)))
    out = np.zeros((S, D), np.float32)
    for rr in res.results:
        out += rr["y"]
    return out



# revision 7
# speedup vs baseline: 1.1225x; 1.1225x over previous
"""Multi-head attention (S=4096, D=1024, H=16) on 8 trn2 NeuronCores.

Sharding: 2 heads per core (tensor-parallel on Q/K/V column splits and
dense row split). Each core computes a partial [S, D] output; host sums
the 8 partials (the unshard step for row-parallel TP).

Per-core design (bf16 operands, fp32 PSUM accumulate):
  The ACT engine's exp over the S*S*2 logits (256 x [128,1024] tiles,
  ~1.04us each) is the hard floor (~266us); everything else is scheduled
  to hide inside it.
  - logits: lt[128t, 2h*512s] = k-block^T q-window per head (2 matmuls,
    1024 rows).
  - PV streams v, not P: stationary = pt s-block [128t,128s], moving =
    v_aug[128t,65] -> acc[128s, 65] accumulated over 32 t-blocks. 65
    rows/matmul instead of 512 (weight loads are free in the cost
    model); column 64 of v_aug is ones so the softmax denominator lands
    in acc[:,...,64] already laid out per s-partition.
  - normalization: reciprocal of the denominator then a per-partition
    tensor_scalar_mul while copying acc out of PSUM; the old
    K=1-broadcast-matmul normalization path is gone.
  - v-projection emits v directly in [s-part, dk] layout (stationary =
    x-block, moving = Wv chunk), killing the PE transposes of V.
  - output projection: per s-block, transpose normalized attn
    ([128s,64]->[64,128] via PE, 1.0 cyc/row in bf16), then
    yp[128s,512d] = attnT^T wo.
  PSUM (8 banks): lt 2x2 (ping-pong), acc 1x2 (padded [128,2,4,128],
  single-buffered across waves), 2-slot scratch ring shared by proj
  psum / transposes / yp.
  Schedule: deadline-driven weave of projection work into the PE slack
  under ACT; PV lag 20 in wave 0 (protects ACT from the proj burst),
  4 afterwards; warm-up matmuls ramp the PE p-state during the initial
  x DMA; DMA issue order puts wk/wq/x-piece-0 first (HWDGE serializes
  at ~625ns per dma_start).
"""

import numpy as np
from collections import deque
from contextlib import ExitStack

S = 4096
D = 1024
NCORES = 8
HD = 128  # head-dim span per core (2 heads x 64)
DK = 64
NT = S // 128   # 32 t-blocks
NWV = 8         # s-waves of 512
LAG_W0 = 20
LAG = 4
N_WARM = 26

_NC_CACHE = {}


def _split_multi_waits(nc, mybir):
    """This walrus build encodes at most ~2 sync commands per instruction
    (1 for matmul/drain). Keep <=1 wait on every compute/DMA instruction and
    move the rest into standalone dual-condition EventSemaphore instructions
    inserted immediately before it on the same engine (same wait point, so
    semantics are unchanged)."""
    n = 0
    used = set()
    for b in nc.m.functions[0].blocks:
        for inst in b.instructions:
            si = inst.sync_info
            if si:
                for w in (si.on_wait or []):
                    used.add(w.id)
                for u in (si.on_update or []):
                    used.add(u.id)
    free_ids = [i for i in range(max(used) + 1, max(used) + 32)]
    sems = {}

    def eng_sem(eng):
        if eng not in sems:
            sems[eng] = (free_ids.pop(0), f"wsplit_{len(sems)}")
        return sems[eng]

    for b in nc.m.functions[0].blocks:
        il = b.instructions
        new = []
        for inst in il:
            si = inst.sync_info
            waits = list(si.on_wait) if si and si.on_wait else []
            upds = list(si.on_update) if si and si.on_update else []
            if type(inst).__name__ == "InstEventSemaphore":
                new.append(inst)
                continue
            if len(waits) > 1:
                excess, keep = waits[:-1], waits[-1:]
                for i in range(0, len(excess), 2):
                    sid, sname = eng_sem(inst.engine)
                    ev = mybir.InstEventSemaphore(
                        name=f"{inst.name}_ws{i}", engine=inst.engine,
                        ins=[], outs=[],
                        sync_info=mybir.SyncInfo(
                            on_wait=excess[i:i + 2],
                            on_update=[mybir.SyncUpdate(
                                sync_type="semaphore", id=sid,
                                ant_name=sname, update_mode="sem-inc",
                                update_value=1, update_reg=None)]))
                    new.append(ev)
                    n += 1
                inst.sync_info = mybir.SyncInfo(on_wait=keep, on_update=upds)
            new.append(inst)
        il[:] = new
    return n


def _build_nc():
    import concourse.bass as bass
    import concourse.tile as tile
    import concourse.mybir as mybir

    fp32 = mybir.dt.float32
    bf16 = mybir.dt.bfloat16
    AF = mybir.ActivationFunctionType

    nc = bass.Bass()
    xT = nc.dram_tensor("xT", [D, S], bf16, kind="ExternalInput")
    wqT = nc.dram_tensor("wqT", [D, HD], bf16, kind="ExternalInput")
    wkT = nc.dram_tensor("wkT", [D, HD], bf16, kind="ExternalInput")
    wvT = nc.dram_tensor("wvT", [D, HD], bf16, kind="ExternalInput")
    woT = nc.dram_tensor("woT", [HD, D], bf16, kind="ExternalInput")
    ident = nc.dram_tensor("ident", [128, 128], bf16, kind="ExternalInput")
    y = nc.dram_tensor("y", [S, D], fp32, kind="ExternalOutput")

    with tile.TileContext(nc) as tc, ExitStack() as ctx, \
         nc.allow_low_precision(reason="bf16 operands within rel-err budget"):
        sb = ctx.enter_context(tc.tile_pool(name="sb", bufs=1))
        qT_sb = sb.tile([128, S], bf16, tag="qT")
        kT_sb = sb.tile([128, S], bf16, tag="kT")
        # v_aug[:, tb, h, 0:64] = v block for head h; [..., 64] = ones
        v_aug = sb.tile([128, NT, 2, DK + 1], bf16, tag="vaug")
        wq_sb = sb.tile([128, 8, HD], bf16, tag="wq")
        wk_sb = sb.tile([128, 8, HD], bf16, tag="wk")
        wv_sb = sb.tile([128, 8, HD], bf16, tag="wv")
        wo_sb = sb.tile([HD, D], bf16, tag="wo")
        id_sb = sb.tile([128, 128], bf16, tag="id")
        warm = sb.tile([1, DK], fp32, tag="warm")

        xpool = ctx.enter_context(tc.tile_pool(name="xpool", bufs=8))
        ptpool = ctx.enter_context(tc.tile_pool(name="ptpool", bufs=LAG_W0 + 8))
        attnpool = ctx.enter_context(tc.tile_pool(name="attnpool", bufs=4))
        atTpool = ctx.enter_context(tc.tile_pool(name="atTpool", bufs=2))
        yopool = ctx.enter_context(tc.tile_pool(name="yopool", bufs=3))
        dpool = ctx.enter_context(tc.tile_pool(name="dpool", bufs=2))
        ltpool = ctx.enter_context(
            tc.tile_pool(name="ltpool", bufs=2, space="PSUM"))
        accpool = ctx.enter_context(
            tc.tile_pool(name="accpool", bufs=1, space="PSUM"))
        pspool = ctx.enter_context(
            tc.tile_pool(name="pspool", bufs=2, space="PSUM"))

        # ---- startup DMAs: critical path (wk, wq, x piece 0) first ----
        nc.sync.dma_start(id_sb[:], ident[:])
        nc.sync.dma_start(wk_sb[:], wkT.rearrange("(c p) m -> p c m", p=128))
        nc.sync.dma_start(wq_sb[:], wqT.rearrange("(c p) m -> p c m", p=128))
        xTr = xT.rearrange("(c p) s -> p c s", p=128)
        xqs = {}

        def load_xq(p):
            xq = xpool.tile([128, 8, 512], bf16, tag="xq", name=f"xq_{p}")
            for c in range(0, 8, 2):
                nc.sync.dma_start(xq[:, c:c + 2, :],
                                  xTr[:, c:c + 2, p * 512:(p + 1) * 512])
            xqs[p] = xq

        load_xq(0)
        nc.sync.dma_start(wv_sb[:], wvT.rearrange("(c p) m -> p c m", p=128))
        nc.sync.dma_start(wo_sb[:], woT[:])
        nc.vector.memset(v_aug[:, :, :, DK], 1.0)
        # preload the exp table on ACT before the first real exp
        nc.scalar.activation(warm[:], id_sb[0:1, 0:DK], AF.Exp, scale=0.125)
        # warm-up matmuls: ramp the PE p-state while x piece 0 streams in
        for i in range(N_WARM):
            wp = pspool.tile([128, 128], fp32, tag="ps", name=f"warm_{i}")
            nc.tensor.matmul(wp[:], id_sb[:], id_sb[:], start=True, stop=True)
        # remaining x pieces; all 8 stay resident (q-proj of piece p runs
        # ~4 waves after its k-proj, so slots can't rotate)
        for p in range(1, 8):
            load_xq(p)

        # ---- projection pieces (as resumable thunk pairs) ----
        def qk_proj_thunks(p, w_sb, dst):
            """Two thunks: chunks 0-3 and 4-7 of dst[:, p*512:(p+1)*512]."""
            hold = {}

            def first():
                hold["ps"] = pspool.tile([128, 512], fp32, tag="ps",
                                         name=f"pqk_{dst.tensor.name}_{p}")
                for c in range(4):
                    nc.tensor.matmul(hold["ps"][:], w_sb[:, c, :],
                                     xqs[p][:, c, :], start=(c == 0),
                                     stop=False)

            def second():
                pk = hold.pop("ps")
                for c in range(4, 8):
                    nc.tensor.matmul(pk[:], w_sb[:, c, :], xqs[p][:, c, :],
                                     start=False, stop=(c == 7))
                nc.vector.tensor_copy(dst[:, p * 512:(p + 1) * 512], pk[:])
            return [first, second]

        def v_block_thunk(tb):
            def run():
                p, i = tb // 4, tb % 4
                vps = pspool.tile([128, 128], fp32, tag="ps",
                                  name=f"vps_{tb}")
                for c in range(8):
                    nc.tensor.matmul(
                        vps[:], xqs[p][:, c, i * 128:(i + 1) * 128],
                        wv_sb[:, c, :], start=(c == 0), stop=(c == 7))
                nc.vector.tensor_copy(
                    v_aug[:, tb, :, 0:DK],
                    vps[:].rearrange("p (h d) -> p h d", h=2))
            return run

        # ---- startup projections: k piece 0, q piece 0 (feeds lt(0,0)) ----
        for t in qk_proj_thunks(0, wk_sb, kT_sb):
            t()
        for t in qk_proj_thunks(0, wq_sb, qT_sb):
            t()

        # deadline-ordered weave of the remaining projection work
        proj_q = deque()
        for p in range(1, 8):
            a, b = qk_proj_thunks(p, wk_sb, kT_sb)
            proj_q.append([4 * p - 3, a])
            proj_q.append([4 * p - 2, b])
        for tb in range(NT):
            proj_q.append([LAG_W0 + tb - 2, v_block_thunk(tb)])
        for w in range(1, NWV):
            a, b = qk_proj_thunks(w, wq_sb, qT_sb)
            proj_q.append([32 * w - 4, a])
            proj_q.append([32 * w - 3, b])
        proj_q = deque(sorted(proj_q, key=lambda e: e[0]))

        # ---- attention machinery ----
        pending = deque()  # (ready_gi, thunk)
        state = {"gi": 0}
        lt_holder = {}

        def emit_lt(w, tb):
            s0 = w * 512
            lt = ltpool.tile([128, 1024], fp32, tag="lt",
                             name=f"lt_{w}_{tb}")
            for h in range(2):
                nc.tensor.matmul(
                    lt[:, h * 512:(h + 1) * 512],
                    kT_sb[DK * h:DK * (h + 1), tb * 128:(tb + 1) * 128],
                    qT_sb[DK * h:DK * (h + 1), s0:s0 + 512],
                    start=True, stop=True,
                    tile_position=(DK * h, 0),
                )
            return lt

        def pv_thunk(w, tb, pt, accs):
            def run():
                if tb == 0:
                    accs.append(accpool.tile([128, 2, 4, 128], fp32,
                                             tag="acc", name=f"acc_{w}"))
                acc = accs[0]
                for h in range(2):
                    for k in range(4):
                        # one accumulation group per PSUM zero-region (the
                        # 2KB bank holding all 4 k-slices of head h): start
                        # marks the whole bank pending-zero, so only the
                        # first slice may start and only the last may stop;
                        # the other tb==0 writes zero-on-first-touch.
                        nc.tensor.matmul(
                            acc[:, h, k, 0:DK + 1],
                            pt[:, h * 512 + k * 128:h * 512 + (k + 1) * 128],
                            v_aug[:, tb, h, :],
                            start=(tb == 0 and k == 0),
                            stop=(tb == NT - 1 and k == 3),
                            skip_group_check=(k != 0),
                        )
            return run

        def finalize_thunks(w, accs):
            rden = dpool.tile([128, 2, 4], fp32, tag="rden", name=f"rden_{w}")
            attn = {}
            atT = {}

            def recip():
                nc.vector.reciprocal(rden[:], accs[0][:, :, :, DK])

            def scale(h, k):
                a = attnpool.tile([128, DK], bf16, tag="attn",
                                  name=f"attn_{w}_{h}_{k}")
                nc.vector.tensor_scalar_mul(
                    a[:], accs[0][:, h, k, 0:DK], rden[:, h, k:k + 1])
                attn[(h, k)] = a

            def transp(h, k):
                if h == 0:
                    atT[k] = atTpool.tile([128, 128], bf16, tag="atT",
                                          name=f"atT_{w}_{k}")
                tps = pspool.tile([DK, 128], bf16, tag="ps",
                                  name=f"tps_{w}_{h}_{k}")
                nc.tensor.transpose(tps[:], attn.pop((h, k))[:], id_sb[:])
                nc.vector.tensor_copy(atT[k][DK * h:DK * (h + 1), :], tps[:])

            def yblock(k, jc):
                b = w * 4 + k
                yp = pspool.tile([128, 512], fp32, tag="ps",
                                 name=f"yp_{b}_{jc}")
                nc.tensor.matmul(
                    yp[:], atT[k][:], wo_sb[:, jc * 512:(jc + 1) * 512],
                    start=True, stop=True)
                yo = yopool.tile([128, 512], fp32, tag="yo",
                                 name=f"yo_{b}_{jc}")
                nc.vector.tensor_copy(yo[:], yp[:])
                nc.sync.dma_start(
                    y[b * 128:(b + 1) * 128, jc * 512:(jc + 1) * 512],
                    yo[:])

            thunks = [recip]
            for k in range(4):
                thunks.append(lambda k=k: (scale(0, k), scale(1, k)))
                thunks.append(lambda k=k: (transp(0, k), transp(1, k)))
                for jc in range(2):
                    thunks.append(lambda k=k, jc=jc: yblock(k, jc))
            return thunks

        def emit_iter(w, tb, accs):
            gi = state["gi"]
            lag = LAG_W0 if w == 0 else LAG
            lt = lt_holder.pop("lt")
            pt = ptpool.tile([128, 1024], bf16, tag="pt",
                             name=f"pt_{w}_{tb}")
            nc.scalar.activation(pt[:], lt[:], AF.Exp, scale=0.125)
            if tb + 1 < NT:
                lt_holder["lt"] = emit_lt(w, tb + 1)
            elif w + 1 < NWV:
                lt_holder["lt"] = emit_lt(w + 1, 0)
            pending.append((gi + lag, pv_thunk(w, tb, pt, accs)))
            pops = 0
            while pending and pending[0][0] <= gi and pops < 2:
                pending.popleft()[1]()
                pops += 1
            pops = 0
            while proj_q and proj_q[0][0] <= gi and pops < 2:
                proj_q.popleft()[1]()
                pops += 1
            state["gi"] = gi + 1

        # ---- main loop ----
        lt_holder["lt"] = emit_lt(0, 0)
        for w in range(NWV):
            accs = []
            for tb in range(NT):
                emit_iter(w, tb, accs)
            for j, t in enumerate(finalize_thunks(w, accs)):
                pending.append((state["gi"] + LAG - 1 + j // 2, t))
        while proj_q:
            proj_q.popleft()[1]()
        while pending:
            pending.popleft()[1]()

    _split_multi_waits(nc, mybir)
    nc.finalize()
    return nc


def _get_nc():
    if "nc" not in _NC_CACHE:
        _NC_CACHE["nc"] = _build_nc()
    return _NC_CACHE["nc"]


def _in_maps(x, Wq, Wk, Wv, Wo):
    import ml_dtypes
    bf16 = ml_dtypes.bfloat16
    xT = np.ascontiguousarray(np.asarray(x, np.float32).T).astype(bf16)
    ident = np.eye(128, dtype=np.float32).astype(bf16)
    maps = []
    for c in range(NCORES):
        sl = slice(HD * c, HD * (c + 1))
        maps.append(dict(
            xT=xT,
            wqT=np.ascontiguousarray(np.asarray(Wq)[sl, :].T).astype(bf16),
            wkT=np.ascontiguousarray(np.asarray(Wk)[sl, :].T).astype(bf16),
            wvT=np.ascontiguousarray(np.asarray(Wv)[sl, :].T).astype(bf16),
            woT=np.ascontiguousarray(np.asarray(Wo)[:, sl].T).astype(bf16),
            ident=ident,
        ))
    return maps


def kernel(x, Wq, Wk, Wv, Wo):
    from concourse.bass_utils import run_bass_kernel_spmd

    x = np.asarray(x, dtype=np.float32)
    nc = _get_nc()
    res = run_bass_kernel_spmd(nc, _in_maps(x, Wq, Wk, Wv, Wo),
                               list(range(NCORES)))
    out = np.zeros((S, D), np.float32)
    for rr in res.results:
        out += rr["y"]
    return out
